# revision 33
# baseline (speedup 1.0000x reference)
"""Trainium2 Bass kernel for blocked (sliding-window, non-overlapping) attention.

Reference computation (per batch b):
    q = Wq @ x1 + bq ; k = Wk @ x1 + bk ; v = Wv @ x1 + bv      (1x1 convs)
    split L into blocks of 64; per block: softmax((q^T k)/sqrt(C) masked) @ v^T
    h = relu(attn); out = Wo @ h + bo

Sharding: sequence-parallel over L (blocks are independent): each of the 8
cores gets a contiguous L/8 = 2048 slice of x1/mask for all 4 batches, with
the small conv weights replicated. No collectives needed.

Numerics: v/out/attention matmuls run in bf16 (f32 accumulation). The k
projection (and the low 256 channels of the q projection) run in fp8 e4m3
with DoubleRow perf mode (~1.45x measured PE rate): softmax is insensitive
to small score perturbations here (score std ~0.2), so the fp8 quantization
of x1 and Wk/Wq-low costs ~1.6e-2 relative error end to end vs the 2e-2
gate (simulated and hardware-confirmed). fp8 and bf16 q/k weights are
pre-scaled by 64 host-side (e4m3 min-normal is 2^-6; raw weights sit at
sigma=0.02); since all biases in this problem are zero, the joint descale
(1/64^2) and the 1/sqrt(C) score scale fold into the Exp activation's
scale operand and every projection epilogue is a plain psum->sbuf cast.
(A with-bias graph variant is kept as a fallback and compiled only if the
inputs ever carry nonzero biases.)

Key masking is an additive -30000 (pre-scaled by 64^2 host-side) folded
into the scores psum via a tiny K=2 outer-product matmul; softmax skips
max-subtraction (scores are O(1) by construction). Two 64-blocks are
processed per step as a [128, 128] block-diagonal pair.

Schedule: the q/k/v projections of batch b+1 are emitted interleaved into
batch b's attention loop so the PE never starves on the softmax round-trip;
input DMAs are prefetched an attention-span ahead (x1 on the sync queue,
x8/mask on the scalar queue, out stores on the gpsimd queue — per-queue
transfers serialize, so streams are separated by direction/size). Output
is written bf16 (halves the out DMA) and upcast on the host.
"""

import sys

sys.path.insert(0, "/opt/trn_rl_repo")

import numpy as np
import ml_dtypes

B = 4
C_IN = 512
L = 16384
CR = 256          # reduced (q/k/v) channels
BL = 64           # attention block
N_CORES = 8
LS = L // N_CORES  # 2048 per-core sequence shard
NT = LS // 512     # 4 free-dim tiles of 512
KC = C_IN // 128   # 4 contraction chunks for q/k/v projections
MC = CR // 128     # 2 chunks of reduced channels
OC = C_IN // 128   # 4 chunks of output channels
NPAIR = LS // 128  # 16 block-pairs per batch per core
NEGM = -30000.0
QSCALE = 1.0 / 16.0  # 1/sqrt(C_RED)
WS = 64.0            # fp8/bf16 q,k weight pre-scale (descale folded into Exp)

QMODE = "half"       # 'half': q chunks 0,1 in fp8 DoubleRow; 'bf16': all bf16

_CACHE = {}


def _build_graph(general):
    import concourse.bass as bass
    import concourse.tile as tile
    from concourse import bacc, mybir

    f32 = mybir.dt.float32
    bf16 = mybir.dt.bfloat16
    f8 = mybir.dt.float8e4
    AF = mybir.ActivationFunctionType
    DR = mybir.MatmulPerfMode.DoubleRow

    nc = bacc.Bacc(None, target_bir_lowering=False)

    # host-prearranged layouts: everything DMAs contiguously per partition.
    x1_e = nc.declare_dram_parameter("x1r", [B, 128, KC * LS], bf16, isOutput=False)
    x8_e = nc.declare_dram_parameter("x8r", [B, 128, 4 * LS], f8, isOutput=False)
    wq_e = nc.declare_dram_parameter("wq4", [128, KC * 128 * MC], bf16, isOutput=False)
    wk8_e = nc.declare_dram_parameter("wk8", [128, 4 * CR], f8, isOutput=False)
    wq8_e = nc.declare_dram_parameter("wq8", [128, 2 * CR], f8, isOutput=False)
    wv_e = nc.declare_dram_parameter("wvT", [128, KC * CR], bf16, isOutput=False)
    wo_e = nc.declare_dram_parameter("woT", [128, MC * C_IN], bf16, isOutput=False)
    if general:
        bia_e = nc.declare_dram_parameter(
            "biases", [128, 2 * MC + 2 + OC], f32, isOutput=False)
    if general:
        md_e = nc.declare_dram_parameter("madd2", [B, 2, NPAIR // 2, 256], bf16, isOutput=False)
        on_e = nc.declare_dram_parameter("onesbd", [2, 128], bf16, isOutput=False)
    id_e = nc.declare_dram_parameter("ident", [128, 128], bf16, isOutput=False)
    out_e = nc.declare_dram_parameter("out", [B, C_IN, LS], bf16, isOutput=True)

    PS = bass.MemorySpace.PSUM

    with tile.TileContext(nc) as tc:
        with (
            tc.tile_pool(name="const", bufs=1) as constp,
            tc.tile_pool(name="mdp", bufs=3) as mdp,
            tc.tile_pool(name="x1p", bufs=2) as x1p,
            tc.tile_pool(name="x8p", bufs=2) as x8p,
            tc.tile_pool(name="qkp", bufs=32) as qkp,
            tc.tile_pool(name="vtp", bufs=18) as vtp,
            tc.tile_pool(name="hp", bufs=8) as hp,
            tc.tile_pool(name="outp", bufs=8) as outp,
            tc.tile_pool(name="smp", bufs=10) as smp,
            tc.tile_pool(name="psA", bufs=2, space=PS) as psA,
            tc.tile_pool(name="psV", bufs=1, space=PS) as psV,
            tc.tile_pool(name="psS", bufs=2, space=PS) as psS,
            tc.tile_pool(name="psT", bufs=1, space=PS) as psT,
            tc.tile_pool(name="psAt", bufs=2, space=PS) as psAt,
        ):
            # ---- replicated constants (scalar queue; per-queue transfers
            # serialize, so order = criticality) ----
            wq_sb = constp.tile([128, KC, 128 * MC], bf16, tag="wq")
            nc.scalar.dma_start(wq_sb[:], wq_e[:].rearrange("p (k c) -> p k c", k=KC))

            x1_st = {}   # b -> x1 tile [128, KC, LS] bf16
            x8_st = {}   # b -> x8 tile [128, 2, 2, LS] fp8
            md_st = {}   # b -> madd tile

            def load_b(b):
                x1t = x1p.tile([128, KC, LS], bf16, tag="x1", name=f"x1_{b}")
                x8t = x8p.tile([128, 2, 2, LS], f8, tag="x8", name=f"x8_{b}")
                xv = x1_e[b].rearrange("p (k l) -> p k l", k=KC)
                x8v = x8_e[b].rearrange("p (c t l) -> p c t l", c=2, t=2)
                # big x1 stream on sync; x8/mask ride the scalar queue so
                # they don't queue behind a 2MB transfer.
                nc.sync.dma_start(x1t[:], xv)
                nc.scalar.dma_start(x8t[:], x8v)
                x1_st[b], x8_st[b] = x1t, x8t
                if general:
                    mdt = mdp.tile([2, NPAIR // 2, 256], bf16, tag="md", name=f"md_{b}")
                    nc.scalar.dma_start(mdt[:], md_e[b])
                    md_st[b] = mdt

            # batch 0: head DMAs in exact consumption order, spread over the
            # sync (x1), vector (x8) and scalar (weights) queues so the q/k/v
            # projections are never gated on a single queue's bandwidth.
            x1t0 = x1p.tile([128, KC, LS], bf16, tag="x1", name="x1_0")
            x8t0 = x8p.tile([128, 2, 2, LS], f8, tag="x8", name="x8_0")
            xv0 = x1_e[0].rearrange("p (k l) -> p k l", k=KC)
            x8v0 = x8_e[0].rearrange("p (c t l) -> p c t l", c=2, t=2)
            nc.sync.dma_start(x1t0[:, 2:4, 0:512], xv0[:, 2:4, 0:512])
            nc.sync.dma_start(x1t0[:, 2:4, 512:1024], xv0[:, 2:4, 512:1024])
            x1_st[0], x8_st[0] = x1t0, x8t0
            if general:
                md0 = mdp.tile([2, NPAIR // 2, 256], bf16, tag="md", name="md_0")
                md_st[0] = md0

            wq8_sb = constp.tile([128, 2, CR], f8, tag="wq8")
            nc.scalar.dma_start(wq8_sb[:], wq8_e[:].rearrange("p (t m) -> p t m", t=2))
            nc.gpsimd.dma_start(x8t0[:, 0, :, 0:1024], x8v0[:, 0, :, 0:1024])
            nc.gpsimd.dma_start(x8t0[:, 0, :, 1024:LS], x8v0[:, 0, :, 1024:LS])
            nc.sync.dma_start(x1t0[:, 2:4, 1024:LS], xv0[:, 2:4, 1024:LS])
            nc.gpsimd.dma_start(x8t0[:, 1], x8v0[:, 1])
            nc.sync.dma_start(x1t0[:, 0:2, 0:1024], xv0[:, 0:2, 0:1024])
            nc.sync.dma_start(x1t0[:, 0:2, 1024:LS], xv0[:, 0:2, 1024:LS])
            wk8_sb = constp.tile([128, 2, 2, CR], f8, tag="wk8")
            nc.scalar.dma_start(wk8_sb[:], wk8_e[:].rearrange("p (c t m) -> p c t m", c=2, t=2))
            if general:
                bia_sb = constp.tile([128, 2 * MC + 2 + OC], f32, tag="bia")
                nc.scalar.dma_start(bia_sb[:], bia_e[:])
                bq_sb = bia_sb[:, 0:MC]
                bk_sb = bia_sb[:, MC:2 * MC]
                bv_sb = bia_sb[:, 2 * MC:2 * MC + 2]
                bo_sb = bia_sb[:, 2 * MC + 2:]
            wv_sb = constp.tile([128, KC, CR], bf16, tag="wv")
            nc.scalar.dma_start(wv_sb[:], wv_e[:].rearrange("p (k c) -> p k c", k=KC))
            id_sb = constp.tile([128, 128], bf16, tag="id")
            nc.scalar.dma_start(id_sb[:], id_e[:])
            if general:
                on_sb = constp.tile([2, 128], bf16, tag="on")
                nc.scalar.dma_start(on_sb[:], on_e[:])
            wo_sb = constp.tile([128, MC, C_IN], bf16, tag="wo")
            nc.scalar.dma_start(wo_sb[:], wo_e[:].rearrange("p (m c) -> p m c", m=MC))

            q_sb = {}   # (b, m, t) -> bf16 [128, 512]
            k_sb = {}
            vt_sb = {}  # (b, u) -> bf16 [128, 2, CR]
            h_sb = {}   # (b, t) -> bf16 [128, MC, 4, 128]

            # ---- projection work units (emitted interleaved) ----
            def q_unit(b, m, t):
                x1t, x8t = x1_st[b], x8_st[b]
                msl = slice(m * 128, (m + 1) * 128)
                tsl = slice(t * 512, (t + 1) * 512)
                ps = psA.tile([128, 512], f32, tag="psA", name=f"qps_{b}_{m}_{t}")
                if QMODE == "half":
                    # bf16 chunks first: the very first matmul then only
                    # gates on the small wq + x1[0:512] head DMAs.
                    for kc in (2, 3):
                        nc.tensor.matmul(
                            ps[:], wq_sb[:, kc, msl], x1t[:, kc, tsl],
                            start=(kc == 2), stop=False,
                        )
                    nc.tensor.matmul(
                        ps[:], wq8_sb[:, :, msl], x8t[:, 0, :, tsl],
                        start=False, stop=True, perf_mode=DR,
                    )
                else:
                    for kc in range(KC):
                        nc.tensor.matmul(
                            ps[:], wq_sb[:, kc, msl], x1t[:, kc, tsl],
                            start=(kc == 0), stop=(kc == KC - 1),
                        )
                qt = qkp.tile([128, 512], bf16, tag="qk", name=f"q_{b}_{m}_{t}")
                if general:
                    nc.scalar.activation(
                        qt[:], ps[:], AF.Identity,
                        bias=bq_sb[:, m:m + 1], scale=QSCALE / WS,
                    )
                else:
                    nc.scalar.copy(qt[:], ps[:])
                q_sb[(b, m, t)] = qt

            def k_unit(b, m, t):
                x8t = x8_st[b]
                msl = slice(m * 128, (m + 1) * 128)
                tsl = slice(t * 512, (t + 1) * 512)
                ps = psA.tile([128, 512], f32, tag="psA", name=f"kps_{b}_{m}_{t}")
                for cp in range(2):
                    nc.tensor.matmul(
                        ps[:], wk8_sb[:, cp, :, msl], x8t[:, cp, :, tsl],
                        start=(cp == 0), stop=(cp == 1), perf_mode=DR,
                    )
                kt = qkp.tile([128, 512], bf16, tag="qk", name=f"k_{b}_{m}_{t}")
                if general:
                    nc.vector.tensor_scalar(
                        kt[:], ps[:], 1.0 / WS, bk_sb[:, m:m + 1],
                        op0=mybir.AluOpType.mult, op1=mybir.AluOpType.add,
                    )
                else:
                    nc.scalar.copy(kt[:], ps[:])
                k_sb[(b, m, t)] = kt

            def v_unit(b, u):
                # v^T for one attention step (two 128-l chunks)
                x1t = x1_st[b]
                ps = psV.tile([128, 2, CR], f32, tag="psV", name=f"vps_{b}_{u}")
                for j in range(2):
                    psl = slice((2 * u + j) * 128, (2 * u + j + 1) * 128)
                    for kc in range(KC):
                        nc.tensor.matmul(
                            ps[:, j, :], x1t[:, kc, psl], wv_sb[:, kc, :],
                            start=(j == 0 and kc == 0),
                            stop=(j == 1 and kc == KC - 1),
                        )
                vt = vtp.tile([128, 2, CR], bf16, tag="vt", name=f"vt_{b}_{u}")
                nc.vector.tensor_copy(vt[:], ps[:])
                vt_sb[(b, u)] = vt

            def proj_units(b):
                us = [lambda m=m, t=t: q_unit(b, m, t) for m in range(MC) for t in range(NT)]
                us += [lambda m=m, t=t: k_unit(b, m, t) for m in range(MC) for t in range(NT)]
                us += [lambda u=u: v_unit(b, u) for u in range(NPAIR // 2)]
                return us

            # ---- blocked attention, four 64-blocks (2 pairs) per step ----
            def attn_front(b, u):
                """scores + softmax for pairs (2u, 2u+1); returns probs."""
                t, half = divmod(u, 2)
                w = half * 256
                mdt = md_st.get(b)
                sc = psS.tile([128, 256], f32, tag="sc", name=f"sc_{b}_{u}")
                for u2 in range(2):
                    qsl = slice(w + u2 * 128, w + u2 * 128 + 128)
                    o0 = u2 * 128
                    for m in range(MC):
                        nc.tensor.matmul(
                            sc[:, o0:o0 + 128],
                            q_sb[(b, m, t)][:, qsl], k_sb[(b, m, t)][:, qsl],
                            start=(u2 == 0 and m == 0),
                            stop=(not general and u2 == 1 and m == MC - 1),
                        )
                if general:
                    nc.tensor.matmul(
                        sc[:], on_sb[:], mdt[:, u, :],
                        start=False, stop=True,
                    )
                esc = smp.tile([128, 2, 128], bf16, tag="esc", name=f"esc_{b}_{u}")
                rs = smp.tile([128, 2], f32, tag="rs", name=f"rs_{b}_{u}")
                if general:
                    nc.scalar.activation(esc[:], sc[:], AF.Exp)
                else:
                    # qt/kt carry the 64x weight pre-scale; 1/sqrt(C) and
                    # 1/64^2 fold into the Exp input scale (exact pow2).
                    # Off-diagonal 64x64 quadrants of each pair block carry
                    # cross-block garbage; zero them (== exp of -inf mask).
                    nc.scalar.activation(esc[:], sc[:], AF.Exp, scale=QSCALE / (WS * WS))
                    for u2 in range(2):
                        nc.gpsimd.memset(esc[0:64, u2, 64:128], 0.0)
                        nc.gpsimd.memset(esc[64:128, u2, 0:64], 0.0)
                nc.vector.reduce_sum(out=rs[:], in_=esc[:], axis=mybir.AxisListType.X)
                rc = smp.tile([128, 2], f32, tag="rc", name=f"rc_{b}_{u}")
                nc.vector.reciprocal(rc[:], rs[:])
                pr = smp.tile([128, 2, 128], bf16, tag="pr", name=f"pr_{b}_{u}")
                for u2 in range(2):
                    nc.vector.tensor_scalar_mul(
                        pr[:, u2, :], esc[:, u2, :], rc[:, u2:u2 + 1],
                    )
                return pr

            def attn_back(b, u, pr):
                """transpose + attn + relu for pairs (2u, 2u+1)."""
                t, half = divmod(u, 2)
                prT_ps = psT.tile([128, 256], bf16, tag="prT", name=f"prT_{b}_{u}")
                for u2 in range(2):
                    nc.tensor.transpose(
                        prT_ps[:, u2 * 128:(u2 + 1) * 128], pr[:, u2, :], id_sb[:],
                    )
                prT = smp.tile([128, 256], bf16, tag="prTs", name=f"prTs_{b}_{u}")
                nc.vector.tensor_copy(prT[:], prT_ps[:])
                at = psAt.tile([128, 2, MC, 128], f32, tag="at", name=f"at_{b}_{u}")
                vt = vt_sb[(b, u)]
                for u2 in range(2):
                    for m in range(MC):
                        nc.tensor.matmul(
                            at[:, u2, m, :],
                            vt[:, u2, m * 128:(m + 1) * 128],
                            prT[:, u2 * 128:(u2 + 1) * 128],
                            start=True, stop=True,
                        )
                for m in range(MC):
                    hout = h_sb[(b, t)][:, m, half * 2:half * 2 + 2, :]
                    if general:
                        if m == 0:
                            nc.scalar.activation(
                                hout, at[:, :, m, :], AF.Relu, bias=bv_sb[:, m:m + 1],
                            )
                        else:
                            nc.vector.tensor_scalar(
                                hout, at[:, :, m, :], bv_sb[:, m:m + 1], 0.0,
                                op0=mybir.AluOpType.add, op1=mybir.AluOpType.max,
                            )
                    else:
                        if m == 0:
                            nc.scalar.activation(hout, at[:, :, m, :], AF.Relu)
                        else:
                            nc.vector.tensor_scalar_max(hout, at[:, :, m, :], 0.0)

            def out_proj(b, t):
                # output projection + store for one 512-wide l-tile
                for o in range(OC):
                    osl = slice(o * 128, (o + 1) * 128)
                    ps = psA.tile([128, 512], f32, tag="psA", name=f"ops_{b}_{t}_{o}")
                    for m in range(MC):
                        nc.tensor.matmul(
                            ps[:], wo_sb[:, m, osl], h_sb[(b, t)][:, m, :, :],
                            start=(m == 0), stop=(m == MC - 1),
                        )
                    ot = outp.tile([128, 512], bf16, tag="ot", name=f"ot_{b}_{o}_{t}")
                    if general:
                        if (o + t) % 2 == 0:
                            nc.scalar.activation(
                                ot[:], ps[:], AF.Identity, bias=bo_sb[:, o:o + 1],
                            )
                        else:
                            nc.vector.tensor_scalar_add(ot[:], ps[:], bo_sb[:, o:o + 1])
                    else:
                        if (o + t) % 2 == 0:
                            nc.scalar.copy(ot[:], ps[:])
                        else:
                            nc.vector.tensor_copy(ot[:], ps[:])
                    # out stores ride the (otherwise idle) gpsimd queue so
                    # they never queue behind the big x1/x8 input streams.
                    nc.gpsimd.dma_start(
                        out_e[b, osl, t * 512:(t + 1) * 512], ot[:],
                    )

            def emit_batch(b, units):
                """attention for batch b with next batch's projections woven in."""
                h_sb.update({(b, t): hp.tile([128, MC, 4, 128], bf16, tag="h",
                                             name=f"h_{b}_{t}") for t in range(NT)})
                ui = 0

                def take(n):
                    nonlocal ui
                    stop = min(ui + n, len(units))
                    while ui < stop:
                        units[ui]()
                        ui += 1

                pend = []
                for u in range(NPAIR // 2):
                    pend.append((u, attn_front(b, u)))
                    if len(pend) > 2:
                        pu, ppr = pend.pop(0)
                        attn_back(b, pu, ppr)
                        if pu % 2 == 1:
                            out_proj(b, pu // 2)
                    take(3)
                for pu, ppr in pend:
                    attn_back(b, pu, ppr)
                    if pu % 2 == 1:
                        out_proj(b, pu // 2)
                take(len(units))

            # batch 0's projections run standalone (the head); its remaining
            # input DMAs are issued between phases to bound in-flight count.
            if general:
                nc.scalar.dma_start(md0[:], md_e[0])
            for t in range(NT):
                for m in range(MC):
                    q_unit(0, m, t)
            kv = [lambda m=m, t=t: k_unit(0, m, t) for m in range(MC) for t in range(NT)]
            vs = [lambda u=u: v_unit(0, u) for u in range(NPAIR // 2)]
            kv_i = [f for pair in zip(kv, vs) for f in pair]
            for f in kv_i[:4]:
                f()
            load_b(1)
            for f in kv_i[4:]:
                f()
            for b in range(B):
                if b + 2 < B:
                    load_b(b + 2)  # an attention-span of DMA lead time
                emit_batch(b, proj_units(b + 1) if b + 1 < B else [])

    nc.compile()
    return nc


def _get_graph(general):
    key = ("nc", general)
    if key not in _CACHE:
        _CACHE[key] = _build_graph(general)
    return _CACHE[key]


def _make_in_maps(inputs, general):
    x1 = np.asarray(inputs["x1"])
    mask = np.asarray(inputs["mask"])
    Wq, bq = np.asarray(inputs["Wq"]), np.asarray(inputs["bq"])
    Wk, bk = np.asarray(inputs["Wk"]), np.asarray(inputs["bk"])
    Wv, bv = np.asarray(inputs["Wv"]), np.asarray(inputs["bv"])
    Wo, bo = np.asarray(inputs["Wo"]), np.asarray(inputs["bo"])

    bf16 = ml_dtypes.bfloat16
    e4 = ml_dtypes.float8_e4m3

    def to8(a):
        return np.clip(a, -240.0, 240.0).astype(e4)

    # weight layouts: p-major per 128-channel contraction chunk; q/k copies
    # pre-scaled by WS (descale folded into epilogue/Exp).
    wqT = np.ascontiguousarray(Wq.T) * WS           # [C_IN, CR]
    wkT = np.ascontiguousarray(Wk.T) * WS
    wq4 = wqT.reshape(KC, 128, CR).transpose(1, 0, 2).reshape(128, KC * CR)
    wq4 = np.ascontiguousarray(wq4).astype(bf16)
    # fp8 pair layouts: [p, chunk-pair, pair, out]
    wk8 = wkT.reshape(2, 2, 128, CR).transpose(2, 0, 1, 3).reshape(128, 4 * CR)
    wk8 = to8(np.ascontiguousarray(wk8))
    wq8 = wqT[:CR].reshape(2, 128, CR).transpose(1, 0, 2).reshape(128, 2 * CR)
    wq8 = to8(np.ascontiguousarray(wq8))
    wvT = np.ascontiguousarray(
        Wv.T.reshape(KC, 128, CR).transpose(1, 0, 2).reshape(128, KC * CR)
    ).astype(bf16)
    woT = np.ascontiguousarray(
        Wo.T.reshape(MC, 128, C_IN).transpose(1, 0, 2).reshape(128, MC * C_IN)
    ).astype(bf16)
    ident = np.eye(128, dtype=bf16)
    onesbd = np.zeros((2, 128), dtype=bf16)
    onesbd[0, :64] = 1
    onesbd[1, 64:] = 1

    x1b = x1.astype(bf16)
    if general:
        madd = np.where(mask[:, 0, :] == 0, np.float32(NEGM), np.float32(0.0))
        neg = np.float32(NEGM)

    shared = {
        "wq4": wq4, "wk8": wk8, "wq8": wq8, "wvT": wvT, "woT": woT,
        "ident": ident,
    }
    if general:
        shared["onesbd"] = onesbd
    if general:
        biases = np.concatenate([
            bq.reshape(MC, 128).T,
            bk.reshape(MC, 128).T,
            bv.reshape(MC, 128).T,
            bo.reshape(OC, 128).T,
        ], axis=1).astype(np.float32)
        shared["biases"] = np.ascontiguousarray(biases)
    in_maps = []
    for c in range(N_CORES):
        sl = slice(c * LS, (c + 1) * LS)
        x1s = x1b[:, :, sl]
        # bf16: [b, p, kc, l];  fp8: [b, p, chunk-pair, pair, l]
        x1r = np.ascontiguousarray(
            x1s.reshape(B, KC, 128, LS).transpose(0, 2, 1, 3).reshape(B, 128, KC * LS)
        )
        x8r = to8(np.ascontiguousarray(
            x1s.astype(np.float32).reshape(B, 2, 2, 128, LS)
            .transpose(0, 3, 1, 2, 4).reshape(B, 128, 4 * LS)
        ))
        im = {"x1r": x1r, "x8r": x8r, **shared}
        if general:
            # madd2[b, j, u, :]: additive mask row j for the 2-pair step u.
            m4 = madd[:, sl].reshape(B, NPAIR // 2, 2, 2, 64)
            md2 = np.full((B, 2, NPAIR // 2, 2, 2, 64), neg, np.float32)
            md2[:, 0, :, :, 0, :] = m4[:, :, :, 0, :]
            md2[:, 1, :, :, 1, :] = m4[:, :, :, 1, :]
            im["madd2"] = md2.reshape(B, 2, NPAIR // 2, 256).astype(bf16)
        in_maps.append(im)
    return in_maps


def kernel(**inputs):
    from concourse.bass_utils import run_bass_kernel_spmd

    general = any(
        np.any(np.asarray(inputs[k]) != 0) for k in ("bq", "bk", "bv", "bo")
    ) or bool(np.any(np.asarray(inputs["mask"]) == 0))
    nc = _get_graph(general)
    in_maps = _make_in_maps(inputs, general)
    res = run_bass_kernel_spmd(nc, in_maps, core_ids=list(range(N_CORES)))
    _CACHE["last_results"] = res
    out = np.concatenate(
        [res.results[i]["out"].astype(np.float32) for i in range(N_CORES)], axis=2
    )
    return out


# revision 41
# speedup vs baseline: 1.2791x; 1.2791x over previous
"""Trainium2 Bass kernel for blocked (sliding-window, non-overlapping) attention.

Reference computation (per batch b):
    q = Wq @ x1 + bq ; k = Wk @ x1 + bk ; v = Wv @ x1 + bv      (1x1 convs)
    split L into blocks of 64; per block: softmax((q^T k)/sqrt(C) masked) @ v^T
    h = relu(attn); out = Wo @ h + bo

Sharding: sequence-parallel over L (blocks are independent): each of the 8
cores gets a contiguous L/8 = 2048 slice of x1/mask for all 4 batches, with
the small conv weights replicated. No collectives needed.

Numerics: v/out/attention matmuls run in bf16 (f32 accumulation). The k
projection (and the low 256 channels of the q projection) run in fp8 e4m3
with DoubleRow perf mode (~1.45x measured PE rate): softmax is insensitive
to small score perturbations here (score std ~0.2), so the fp8 quantization
of x1 and Wk/Wq-low costs ~1.6e-2 relative error end to end vs the 2e-2
gate (simulated and hardware-confirmed). fp8 and bf16 q/k weights are
pre-scaled by 64 host-side (e4m3 min-normal is 2^-6; raw weights sit at
sigma=0.02); since all biases in this problem are zero, the joint descale
(1/64^2) and the 1/sqrt(C) score scale fold into the Exp activation's
scale operand and every projection epilogue is a plain psum->sbuf cast.
(A with-bias graph variant is kept as a fallback and compiled only if the
inputs ever carry nonzero biases.)

Key masking is an additive -30000 (pre-scaled by 64^2 host-side) folded
into the scores psum via a tiny K=2 outer-product matmul; softmax skips
max-subtraction (scores are O(1) by construction). Two 64-blocks are
processed per step as a [128, 128] block-diagonal pair.

Schedule: the q/k/v projections of batch b+1 are emitted interleaved into
batch b's attention loop so the PE never starves on the softmax round-trip;
input DMAs are prefetched an attention-span ahead (x1 on the sync queue,
x8/mask on the scalar queue, out stores on the gpsimd queue — per-queue
transfers serialize, so streams are separated by direction/size). Output
is written bf16 (halves the out DMA) and upcast on the host.
"""

import sys

sys.path.insert(0, "/opt/trn_rl_repo")

import numpy as np
import ml_dtypes

B = 4
C_IN = 512
L = 16384
CR = 256          # reduced (q/k/v) channels
BL = 64           # attention block
N_CORES = 8
LS = L // N_CORES  # 2048 per-core sequence shard
NT = LS // 512     # 4 free-dim tiles of 512
KC = C_IN // 128   # 4 contraction chunks for q/k/v projections
MC = CR // 128     # 2 chunks of reduced channels
OC = C_IN // 128   # 4 chunks of output channels
NPAIR = LS // 128  # 16 block-pairs per batch per core
NEGM = -30000.0
QSCALE = 1.0 / 16.0  # 1/sqrt(C_RED)
WS = 64.0            # fp8/bf16 q,k weight pre-scale (descale folded into Exp)

QMODE = "half"       # 'half': q chunks 0,1 in fp8 DoubleRow; 'bf16': all bf16

_CACHE = {}


def _build_graph(general):
    import concourse.bass as bass
    import concourse.tile as tile
    from concourse import bacc, mybir

    f32 = mybir.dt.float32
    bf16 = mybir.dt.bfloat16
    f8 = mybir.dt.float8e4
    AF = mybir.ActivationFunctionType
    DR = mybir.MatmulPerfMode.DoubleRow

    nc = bacc.Bacc(None, target_bir_lowering=False)

    # host-prearranged layouts: everything DMAs contiguously per partition.
    x1_e = nc.declare_dram_parameter("x1r", [B, 128, KC * LS], bf16, isOutput=False)
    x8_e = nc.declare_dram_parameter("x8r", [B, 128, 4 * LS], f8, isOutput=False)
    wq_e = nc.declare_dram_parameter("wq4", [128, KC * 128 * MC], bf16, isOutput=False)
    wk8_e = nc.declare_dram_parameter("wk8", [128, 4 * CR], f8, isOutput=False)
    wq8_e = nc.declare_dram_parameter("wq8", [128, 2 * CR], f8, isOutput=False)
    wv_e = nc.declare_dram_parameter("wvT", [128, KC * CR], bf16, isOutput=False)
    wo_e = nc.declare_dram_parameter("woT", [128, MC * C_IN], bf16, isOutput=False)
    if general:
        bia_e = nc.declare_dram_parameter(
            "biases", [128, 2 * MC + 2 + OC], f32, isOutput=False)
    if general:
        md_e = nc.declare_dram_parameter("madd2", [B, 2, NPAIR // 2, 256], bf16, isOutput=False)
        on_e = nc.declare_dram_parameter("onesbd", [2, 128], bf16, isOutput=False)
    id_e = nc.declare_dram_parameter("ident", [128, 128], bf16, isOutput=False)
    out_e = nc.declare_dram_parameter("out", [B, C_IN, LS], bf16, isOutput=True)

    PS = bass.MemorySpace.PSUM

    with tile.TileContext(nc) as tc:
        with (
            tc.tile_pool(name="const", bufs=1) as constp,
            tc.tile_pool(name="mdp", bufs=3) as mdp,
            tc.tile_pool(name="x1p", bufs=2) as x1p,
            tc.tile_pool(name="x8p", bufs=2) as x8p,
            tc.tile_pool(name="qkp", bufs=32) as qkp,
            tc.tile_pool(name="vtp", bufs=18) as vtp,
            tc.tile_pool(name="hp", bufs=8) as hp,
            tc.tile_pool(name="outp", bufs=8) as outp,
            tc.tile_pool(name="smp", bufs=10) as smp,
            tc.tile_pool(name="psA", bufs=2, space=PS) as psA,
            tc.tile_pool(name="psV", bufs=1, space=PS) as psV,
            tc.tile_pool(name="psS", bufs=2, space=PS) as psS,
            tc.tile_pool(name="psT", bufs=1, space=PS) as psT,
            tc.tile_pool(name="psAt", bufs=2, space=PS) as psAt,
        ):
            # ---- replicated constants (scalar queue; per-queue transfers
            # serialize, so order = criticality) ----
            x1_st = {}   # b -> x1 tile [128, KC, LS] bf16
            x8_st = {}   # b -> x8 tile [128, 2, 2, LS] fp8
            md_st = {}   # b -> madd tile

            def load_b(b):
                x1t = x1p.tile([128, KC, LS], bf16, tag="x1", name=f"x1_{b}")
                x8t = x8p.tile([128, 2, 2, LS], f8, tag="x8", name=f"x8_{b}")
                xv = x1_e[b].rearrange("p (k l) -> p k l", k=KC)
                x8v = x8_e[b].rearrange("p (c t l) -> p c t l", c=2, t=2)
                nc.sync.dma_start(x1t[:], xv)
                nc.sync.dma_start(x8t[:], x8v)
                x1_st[b], x8_st[b] = x1t, x8t
                if general:
                    mdt = mdp.tile([2, NPAIR // 2, 256], bf16, tag="md", name=f"md_{b}")
                    nc.scalar.dma_start(mdt[:], md_e[b])
                    md_st[b] = mdt

            # batch 0: x1 arrives in column chunks on sync, feeding the v and
            # q (bf16-mode, so no x8 dependency) units in consumption order;
            # x8 (needed later, by the k units) is split across scalar/sync.
            x1t0 = x1p.tile([128, KC, LS], bf16, tag="x1", name="x1_0")
            x8t0 = x8p.tile([128, 2, 2, LS], f8, tag="x8", name="x8_0")
            xv0 = x1_e[0].rearrange("p (k l) -> p k l", k=KC)
            x8v0 = x8_e[0].rearrange("p (c t l) -> p c t l", c=2, t=2)
            wv_sb = constp.tile([128, KC, CR], bf16, tag="wv")
            nc.scalar.dma_start(wv_sb[:], wv_e[:].rearrange("p (k c) -> p k c", k=KC))
            for cc in range(4):
                nc.sync.dma_start(
                    x1t0[:, :, cc * 512:(cc + 1) * 512], xv0[:, :, cc * 512:(cc + 1) * 512],
                )
            x1_st[0], x8_st[0] = x1t0, x8t0
            if general:
                md0 = mdp.tile([2, NPAIR // 2, 256], bf16, tag="md", name="md_0")
                md_st[0] = md0

            wq_sb = constp.tile([128, KC, 128 * MC], bf16, tag="wq")
            nc.scalar.dma_start(wq_sb[:], wq_e[:].rearrange("p (k c) -> p k c", k=KC))
            wq8_sb = constp.tile([128, 2, CR], f8, tag="wq8")
            nc.scalar.dma_start(wq8_sb[:], wq8_e[:].rearrange("p (t m) -> p t m", t=2))
            wk8_sb = constp.tile([128, 2, 2, CR], f8, tag="wk8")
            nc.scalar.dma_start(wk8_sb[:], wk8_e[:].rearrange("p (c t m) -> p c t m", c=2, t=2))
            nc.scalar.dma_start(x8t0[:, 0], x8v0[:, 0])
            nc.sync.dma_start(x8t0[:, 1], x8v0[:, 1])
            if general:
                bia_sb = constp.tile([128, 2 * MC + 2 + OC], f32, tag="bia")
                nc.scalar.dma_start(bia_sb[:], bia_e[:])
                bq_sb = bia_sb[:, 0:MC]
                bk_sb = bia_sb[:, MC:2 * MC]
                bv_sb = bia_sb[:, 2 * MC:2 * MC + 2]
                bo_sb = bia_sb[:, 2 * MC + 2:]
            id_sb = constp.tile([128, 128], bf16, tag="id")
            nc.scalar.dma_start(id_sb[:], id_e[:])
            if general:
                on_sb = constp.tile([2, 128], bf16, tag="on")
                nc.scalar.dma_start(on_sb[:], on_e[:])
            wo_sb = constp.tile([128, MC, C_IN], bf16, tag="wo")
            nc.scalar.dma_start(wo_sb[:], wo_e[:].rearrange("p (m c) -> p m c", m=MC))

            q_sb = {}   # (b, m, t) -> bf16 [128, 512]
            k_sb = {}
            vt_sb = {}  # (b, u) -> bf16 [128, 2, CR]
            h_sb = {}   # (b, t) -> bf16 [128, MC, 4, 128]

            # ---- projection work units (emitted interleaved) ----
            def q_unit(b, m, t):
                x1t, x8t = x1_st[b], x8_st[b]
                msl = slice(m * 128, (m + 1) * 128)
                tsl = slice(t * 512, (t + 1) * 512)
                ps = psA.tile([128, 512], f32, tag="psA", name=f"qps_{b}_{m}_{t}")
                if QMODE == "half" and b > 0:
                    # batch 0 stays all-bf16 so the head never waits on x8.
                    for kc in (2, 3):
                        nc.tensor.matmul(
                            ps[:], wq_sb[:, kc, msl], x1t[:, kc, tsl],
                            start=(kc == 2), stop=False,
                        )
                    nc.tensor.matmul(
                        ps[:], wq8_sb[:, :, msl], x8t[:, 0, :, tsl],
                        start=False, stop=True, perf_mode=DR,
                    )
                else:
                    for kc in range(KC):
                        nc.tensor.matmul(
                            ps[:], wq_sb[:, kc, msl], x1t[:, kc, tsl],
                            start=(kc == 0), stop=(kc == KC - 1),
                        )
                qt = qkp.tile([128, 512], bf16, tag="qk", name=f"q_{b}_{m}_{t}")
                if general:
                    nc.scalar.activation(
                        qt[:], ps[:], AF.Identity,
                        bias=bq_sb[:, m:m + 1], scale=QSCALE / WS,
                    )
                else:
                    nc.scalar.copy(qt[:], ps[:])
                q_sb[(b, m, t)] = qt

            def k_unit(b, m, t):
                x8t = x8_st[b]
                msl = slice(m * 128, (m + 1) * 128)
                tsl = slice(t * 512, (t + 1) * 512)
                ps = psA.tile([128, 512], f32, tag="psA", name=f"kps_{b}_{m}_{t}")
                for cp in range(2):
                    nc.tensor.matmul(
                        ps[:], wk8_sb[:, cp, :, msl], x8t[:, cp, :, tsl],
                        start=(cp == 0), stop=(cp == 1), perf_mode=DR,
                    )
                kt = qkp.tile([128, 512], bf16, tag="qk", name=f"k_{b}_{m}_{t}")
                if general:
                    nc.vector.tensor_scalar(
                        kt[:], ps[:], 1.0 / WS, bk_sb[:, m:m + 1],
                        op0=mybir.AluOpType.mult, op1=mybir.AluOpType.add,
                    )
                else:
                    nc.scalar.copy(kt[:], ps[:])
                k_sb[(b, m, t)] = kt

            def v_unit(b, u):
                # v^T for one attention step (two 128-l chunks)
                x1t = x1_st[b]
                ps = psV.tile([128, 2, CR], f32, tag="psV", name=f"vps_{b}_{u}")
                for j in range(2):
                    psl = slice((2 * u + j) * 128, (2 * u + j + 1) * 128)
                    for kc in range(KC):
                        nc.tensor.matmul(
                            ps[:, j, :], x1t[:, kc, psl], wv_sb[:, kc, :],
                            start=(j == 0 and kc == 0),
                            stop=(j == 1 and kc == KC - 1),
                        )
                vt = vtp.tile([128, 2, CR], bf16, tag="vt", name=f"vt_{b}_{u}")
                nc.vector.tensor_copy(vt[:], ps[:])
                vt_sb[(b, u)] = vt

            def proj_units(b):
                us = [lambda m=m, t=t: q_unit(b, m, t) for m in range(MC) for t in range(NT)]
                us += [lambda m=m, t=t: k_unit(b, m, t) for m in range(MC) for t in range(NT)]
                us += [lambda u=u: v_unit(b, u) for u in range(NPAIR // 2)]
                return us

            # ---- blocked attention, four 64-blocks (2 pairs) per step ----
            def attn_front(b, u):
                """scores + softmax for pairs (2u, 2u+1); returns probs."""
                t, half = divmod(u, 2)
                w = half * 256
                mdt = md_st.get(b)
                sc = psS.tile([128, 256], f32, tag="sc", name=f"sc_{b}_{u}")
                for u2 in range(2):
                    qsl = slice(w + u2 * 128, w + u2 * 128 + 128)
                    o0 = u2 * 128
                    for m in range(MC):
                        nc.tensor.matmul(
                            sc[:, o0:o0 + 128],
                            q_sb[(b, m, t)][:, qsl], k_sb[(b, m, t)][:, qsl],
                            start=(u2 == 0 and m == 0),
                            stop=(not general and u2 == 1 and m == MC - 1),
                        )
                if general:
                    nc.tensor.matmul(
                        sc[:], on_sb[:], mdt[:, u, :],
                        start=False, stop=True,
                    )
                esc = smp.tile([128, 2, 128], bf16, tag="esc", name=f"esc_{b}_{u}")
                rs = smp.tile([128, 2], f32, tag="rs", name=f"rs_{b}_{u}")
                if general:
                    nc.scalar.activation(esc[:], sc[:], AF.Exp)
                else:
                    # qt/kt carry the 64x weight pre-scale; 1/sqrt(C) and
                    # 1/64^2 fold into the Exp input scale (exact pow2).
                    # Off-diagonal 64x64 quadrants of each pair block carry
                    # cross-block garbage; zero them (== exp of -inf mask).
                    nc.scalar.activation(esc[:], sc[:], AF.Exp, scale=QSCALE / (WS * WS))
                    for u2 in range(2):
                        nc.gpsimd.memset(esc[0:64, u2, 64:128], 0.0)
                        nc.gpsimd.memset(esc[64:128, u2, 0:64], 0.0)
                nc.vector.reduce_sum(out=rs[:], in_=esc[:], axis=mybir.AxisListType.X)
                rc = smp.tile([128, 2], f32, tag="rc", name=f"rc_{b}_{u}")
                nc.vector.reciprocal(rc[:], rs[:])
                pr = smp.tile([128, 2, 128], bf16, tag="pr", name=f"pr_{b}_{u}")
                for u2 in range(2):
                    nc.vector.tensor_scalar_mul(
                        pr[:, u2, :], esc[:, u2, :], rc[:, u2:u2 + 1],
                    )
                return pr

            def attn_back(b, u, pr):
                """transpose + attn + relu for pairs (2u, 2u+1)."""
                t, half = divmod(u, 2)
                prT_ps = psT.tile([128, 256], bf16, tag="prT", name=f"prT_{b}_{u}")
                for u2 in range(2):
                    nc.tensor.transpose(
                        prT_ps[:, u2 * 128:(u2 + 1) * 128], pr[:, u2, :], id_sb[:],
                    )
                prT = smp.tile([128, 256], bf16, tag="prTs", name=f"prTs_{b}_{u}")
                nc.vector.tensor_copy(prT[:], prT_ps[:])
                at = psAt.tile([128, 2, MC, 128], f32, tag="at", name=f"at_{b}_{u}")
                vt = vt_sb[(b, u)]
                for u2 in range(2):
                    for m in range(MC):
                        nc.tensor.matmul(
                            at[:, u2, m, :],
                            vt[:, u2, m * 128:(m + 1) * 128],
                            prT[:, u2 * 128:(u2 + 1) * 128],
                            start=True, stop=True,
                        )
                for m in range(MC):
                    hout = h_sb[(b, t)][:, m, half * 2:half * 2 + 2, :]
                    if general:
                        if m == 0:
                            nc.scalar.activation(
                                hout, at[:, :, m, :], AF.Relu, bias=bv_sb[:, m:m + 1],
                            )
                        else:
                            nc.vector.tensor_scalar(
                                hout, at[:, :, m, :], bv_sb[:, m:m + 1], 0.0,
                                op0=mybir.AluOpType.add, op1=mybir.AluOpType.max,
                            )
                    else:
                        nc.vector.tensor_scalar_max(hout, at[:, :, m, :], 0.0)

            def out_proj(b, t):
                # output projection + store for one 512-wide l-tile
                for o in range(OC):
                    osl = slice(o * 128, (o + 1) * 128)
                    ps = psA.tile([128, 512], f32, tag="psA", name=f"ops_{b}_{t}_{o}")
                    for m in range(MC):
                        nc.tensor.matmul(
                            ps[:], wo_sb[:, m, osl], h_sb[(b, t)][:, m, :, :],
                            start=(m == 0), stop=(m == MC - 1),
                        )
                    ot = outp.tile([128, 512], bf16, tag="ot", name=f"ot_{b}_{o}_{t}")
                    if general:
                        if (o + t) % 2 == 0:
                            nc.scalar.activation(
                                ot[:], ps[:], AF.Identity, bias=bo_sb[:, o:o + 1],
                            )
                        else:
                            nc.vector.tensor_scalar_add(ot[:], ps[:], bo_sb[:, o:o + 1])
                    else:
                        if (o + t) % 2 == 0:
                            nc.scalar.copy(ot[:], ps[:])
                        else:
                            nc.vector.tensor_copy(ot[:], ps[:])
                    # out stores alternate between the gpsimd (SWDGE, ~70GB/s)
                    # and sync queues; the last batch (no more input prefetch
                    # contention) drains entirely on the faster sync queue.
                    if b == B - 1 or o % 2 == 1:
                        nc.sync.dma_start(out_e[b, osl, t * 512:(t + 1) * 512], ot[:])
                    else:
                        nc.gpsimd.dma_start(out_e[b, osl, t * 512:(t + 1) * 512], ot[:])

            def emit_batch(b, units):
                """attention for batch b with next batch's projections woven in."""
                h_sb.update({(b, t): hp.tile([128, MC, 4, 128], bf16, tag="h",
                                             name=f"h_{b}_{t}") for t in range(NT)})
                ui = 0

                def take(n):
                    nonlocal ui
                    stop = min(ui + n, len(units))
                    while ui < stop:
                        units[ui]()
                        ui += 1

                pend = []
                for u in range(NPAIR // 2):
                    pend.append((u, attn_front(b, u)))
                    if len(pend) > 2:
                        pu, ppr = pend.pop(0)
                        attn_back(b, pu, ppr)
                        if pu % 2 == 1:
                            out_proj(b, pu // 2)
                    take(3)
                for pu, ppr in pend:
                    attn_back(b, pu, ppr)
                    if pu % 2 == 1:
                        out_proj(b, pu // 2)
                take(len(units))

            # batch 0's projections run standalone (the head), v/q interleaved
            # in x1 column-chunk consumption order, k (gated on x8) last.
            if general:
                nc.scalar.dma_start(md0[:], md_e[0])
            for cc in range(4):
                v_unit(0, 2 * cc)
                v_unit(0, 2 * cc + 1)
                q_unit(0, 0, cc)
                q_unit(0, 1, cc)
            load_b(1)
            for m in range(MC):
                for t in range(NT):
                    k_unit(0, m, t)
            for b in range(B):
                if b + 2 < B:
                    load_b(b + 2)  # an attention-span of DMA lead time
                emit_batch(b, proj_units(b + 1) if b + 1 < B else [])

    nc.compile()
    return nc


def _get_graph(general):
    key = ("nc", general)
    if key not in _CACHE:
        _CACHE[key] = _build_graph(general)
    return _CACHE[key]


def _make_in_maps(inputs, general):
    x1 = np.asarray(inputs["x1"])
    mask = np.asarray(inputs["mask"])
    Wq, bq = np.asarray(inputs["Wq"]), np.asarray(inputs["bq"])
    Wk, bk = np.asarray(inputs["Wk"]), np.asarray(inputs["bk"])
    Wv, bv = np.asarray(inputs["Wv"]), np.asarray(inputs["bv"])
    Wo, bo = np.asarray(inputs["Wo"]), np.asarray(inputs["bo"])

    bf16 = ml_dtypes.bfloat16
    e4 = ml_dtypes.float8_e4m3

    def to8(a):
        return np.clip(a, -240.0, 240.0).astype(e4)

    # weight layouts: p-major per 128-channel contraction chunk; q/k copies
    # pre-scaled by WS (descale folded into epilogue/Exp).
    wqT = np.ascontiguousarray(Wq.T) * WS           # [C_IN, CR]
    wkT = np.ascontiguousarray(Wk.T) * WS
    wq4 = wqT.reshape(KC, 128, CR).transpose(1, 0, 2).reshape(128, KC * CR)
    wq4 = np.ascontiguousarray(wq4).astype(bf16)
    # fp8 pair layouts: [p, chunk-pair, pair, out]
    wk8 = wkT.reshape(2, 2, 128, CR).transpose(2, 0, 1, 3).reshape(128, 4 * CR)
    wk8 = to8(np.ascontiguousarray(wk8))
    wq8 = wqT[:CR].reshape(2, 128, CR).transpose(1, 0, 2).reshape(128, 2 * CR)
    wq8 = to8(np.ascontiguousarray(wq8))
    wvT = np.ascontiguousarray(
        Wv.T.reshape(KC, 128, CR).transpose(1, 0, 2).reshape(128, KC * CR)
    ).astype(bf16)
    woT = np.ascontiguousarray(
        Wo.T.reshape(MC, 128, C_IN).transpose(1, 0, 2).reshape(128, MC * C_IN)
    ).astype(bf16)
    ident = np.eye(128, dtype=bf16)
    onesbd = np.zeros((2, 128), dtype=bf16)
    onesbd[0, :64] = 1
    onesbd[1, 64:] = 1

    x1b = x1.astype(bf16)
    if general:
        madd = np.where(mask[:, 0, :] == 0, np.float32(NEGM), np.float32(0.0))
        neg = np.float32(NEGM)

    shared = {
        "wq4": wq4, "wk8": wk8, "wq8": wq8, "wvT": wvT, "woT": woT,
        "ident": ident,
    }
    if general:
        shared["onesbd"] = onesbd
    if general:
        biases = np.concatenate([
            bq.reshape(MC, 128).T,
            bk.reshape(MC, 128).T,
            bv.reshape(MC, 128).T,
            bo.reshape(OC, 128).T,
        ], axis=1).astype(np.float32)
        shared["biases"] = np.ascontiguousarray(biases)
    in_maps = []
    for c in range(N_CORES):
        sl = slice(c * LS, (c + 1) * LS)
        x1s = x1b[:, :, sl]
        # bf16: [b, p, kc, l];  fp8: [b, p, chunk-pair, pair, l]
        x1r = np.ascontiguousarray(
            x1s.reshape(B, KC, 128, LS).transpose(0, 2, 1, 3).reshape(B, 128, KC * LS)
        )
        x8r = to8(np.ascontiguousarray(
            x1s.astype(np.float32).reshape(B, 2, 2, 128, LS)
            .transpose(0, 3, 1, 2, 4).reshape(B, 128, 4 * LS)
        ))
        im = {"x1r": x1r, "x8r": x8r, **shared}
        if general:
            # madd2[b, j, u, :]: additive mask row j for the 2-pair step u.
            m4 = madd[:, sl].reshape(B, NPAIR // 2, 2, 2, 64)
            md2 = np.full((B, 2, NPAIR // 2, 2, 2, 64), neg, np.float32)
            md2[:, 0, :, :, 0, :] = m4[:, :, :, 0, :]
            md2[:, 1, :, :, 1, :] = m4[:, :, :, 1, :]
            im["madd2"] = md2.reshape(B, 2, NPAIR // 2, 256).astype(bf16)
        in_maps.append(im)
    return in_maps


def kernel(**inputs):
    from concourse.bass_utils import run_bass_kernel_spmd

    general = any(
        np.any(np.asarray(inputs[k]) != 0) for k in ("bq", "bk", "bv", "bo")
    ) or bool(np.any(np.asarray(inputs["mask"]) == 0))
    nc = _get_graph(general)
    in_maps = _make_in_maps(inputs, general)
    res = run_bass_kernel_spmd(nc, in_maps, core_ids=list(range(N_CORES)))
    _CACHE["last_results"] = res
    out = np.concatenate(
        [res.results[i]["out"].astype(np.float32) for i in range(N_CORES)], axis=2
    )
    return out
